# revision 1
# baseline (speedup 1.0000x reference)
"""GQA (grouped-query attention) Trainium2 kernel, 8-core SPMD.

Problem: B=4, T=2048, d_model=2048, 32 Q heads, 8 KV heads, d_k=64, causal.
Sharding: core = (batch b, half-of-KV-heads h): 8 cores = 4 batches x 2 halves.
Each core computes its 4 KV heads (16 Q heads) for its batch and the partial
output o_half @ Wo_half (row-parallel Wo); host sums the two halves per batch
and adds bo.

Device-side design (per core):
  - x^T resident in SBUF (bf16); K^T and V projections run c-outer in waves
    interleaved with the input DMA stream so the PE never waits on a full
    tensor load (keeps the HAM clock-gate warm from the start).
  - Scores are computed transposed (s^T[tk, tq]) with a FULL K=128 stationary
    operand: the kT chunk holds two KV heads' subspaces; the query tile is
    stored zero-padded (its off-head 64 partitions are 0), so the extra rows
    contribute nothing.  Full 128-row LDWEIGHTS go through the PE background
    weight buffer and hide behind the previous matmul; 64-row row_grp loads
    do not (measured +95ns per score matmul in the previous revision).
  - Q heads are permuted host-side (Wq columns / bq / Wo rows) so each Q-proj
    PSUM chunk holds one even-parity-KV head (rows 0-63) and one odd-parity
    head (rows 64-127): evictions into the zero-padded buffer never shift
    partitions, and each head's rows line up with its KV head's kT rows.
  - Causal restriction: diagonal score tiles only compute/exp columns
    [128*di, 512); the skipped pT region is memset to 0 and only the 128-wide
    triangle block is multiplied by the mask.
  - exp on ACT (no max subtraction: |scores| small here); V carries a ones
    column so PV also yields the softmax denominators; per head one DVE
    reciprocal + GpSimd broadcast + fused divide-evict.
  - Software pipelining: O-projection of tile j-1 and Q-projection of tile
    j+1 are paced between the attention chunks of tile j.
"""

import numpy as np
import ml_dtypes
from contextlib import ExitStack

B, T, D = 4, 2048, 2048
NKV, NREP, DK = 8, 4, 64
HALF_KV = 4                  # kv heads per core
NQH = HALF_KV * NREP         # 16 q heads per core
QD = NQH * DK                # 1024 q dims per core
KVD = HALF_KV * DK           # 256 kv dims per core
NCORES = 8
CD = D // 128                # 16 contraction chunks over d_model
CT = T // 128                # 16 token chunks of 128
TQ = 512                     # query tile width
NTQ = T // TQ                # 4 query tiles
SCALE = 1.0 / np.sqrt(DK)

BF16 = ml_dtypes.bfloat16

# head slot permutation: slot 2m holds an even-parity-KV head (kT rows 0-63),
# slot 2m+1 an odd-parity one (rows 64-127).
SLOT_A = [0, 1, 2, 3, 8, 9, 10, 11]      # kv 0 or 2 -> rows 0-63
SLOT_B = [4, 5, 6, 7, 12, 13, 14, 15]    # kv 1 or 3 -> rows 64-127
HEAD_OF_SLOT = []
for _m in range(8):
    HEAD_OF_SLOT += [SLOT_A[_m], SLOT_B[_m]]
KV_OF_SLOT = [h // NREP for h in HEAD_OF_SLOT]
# q-dim permutation (within this core's 1024 q dims)
QPERM = np.concatenate([np.arange(h * DK, (h + 1) * DK) for h in HEAD_OF_SLOT])

_cache = {}


def _body(ctx, tc, aps):
    import concourse.mybir as mybir
    from concourse.bass import ts, ds

    nc = tc.nc
    f32 = mybir.dt.float32
    bf16 = mybir.dt.bfloat16
    xT, Wq, bqv, Wk, bkv, Wv, bv, Wo, out = (
        aps["xT"], aps["Wq"], aps["bq"], aps["Wk"], aps["bk"], aps["Wv"],
        aps["bv"], aps["Wo"], aps["out"])

    # ---- pools ----------------------------------------------------------
    rp = ctx.enter_context(tc.tile_pool(name="res", bufs=1))
    op = ctx.enter_context(tc.tile_pool(name="ot", bufs=2))
    ptp = ctx.enter_context(tc.tile_pool(name="pt", bufs=3))
    dvp = ctx.enter_context(tc.tile_pool(name="dv", bufs=2))
    wp = ctx.enter_context(tc.tile_pool(name="wk", bufs=2))
    pp = ctx.enter_context(tc.tile_pool(name="ps", bufs=5, space="PSUM"))
    po = ctx.enter_context(tc.tile_pool(name="po", bufs=3, space="PSUM"))

    # ---- resident tiles -------------------------------------------------
    xT_sb = rp.tile([128, CD, T], bf16, tag="xT")           # 64 KiB/part
    Wq_sb = rp.tile([128, CD, QD], bf16, tag="Wq")          # 32 KiB/part
    Wo_sb = rp.tile([128, QD // 128, D], bf16, tag="Wo")    # 32 KiB/part
    kT_sb = rp.tile([128, KVD // 128, T], bf16, tag="kT")   # 8 KiB/part
    v_sb = rp.tile([128, CT, HALF_KV, DK + 1], bf16, tag="v")
    bq_sb = rp.tile([128, QD // 128], f32, tag="bq")
    bk_sb = rp.tile([128, KVD // 128], f32, tag="bk")
    bv_sb = rp.tile([1, KVD], bf16, tag="bv")
    ones_b = rp.tile([1, 128], bf16, tag="ones_b")
    # causal triangle for the partially-masked 128-col block of a diagonal
    # chunk: tri[p, t] = (t >= p)
    tri = rp.tile([128, 128], bf16, tag="tri")
    # U holds the two RESIDENT zero-padded query buffers (tile j uses
    # U[:, j%2]): slot s of a buffer is a [128, 512] slice whose rows
    # (s%2)*64..+64 hold head HEAD_OF_SLOT[s]'s q^T, other rows zero.  The
    # zero halves are memset exactly once -- resident tiles avoid the pool
    # rotation that forced a re-memset per tile (the scheduler sank those
    # onto the critical path at every tile handoff).  During startup,
    # buffer 0 also hosts Wk/Wv: chunk c of Wk at free offset 512c, Wv at
    # 512c+256; the Q-proj evictions overwrite them once K/V-proj is done.
    U = rp.tile([128, 2, NQH * TQ], bf16, tag="qtz")        # 32 KiB/part
    nc.vector.memset(U[:, 1, :], 0.0)

    # ---- DMA order: small constants, then per-chunk xT+Wk+Wv (feeds the
    # c-outer projection waves), then Wq, then Wo.
    for c in range(KVD // 128):
        nc.sync.dma_start(bk_sb[:, c:c + 1], bkv[c, :].unsqueeze(-1))
    nc.sync.dma_start(bv_sb[:, :], bv[:, :])
    for c in range(QD // 128):
        nc.sync.dma_start(bq_sb[:, c:c + 1], bqv[c, :].unsqueeze(-1))
    for c in range(CD):
        nc.sync.dma_start(xT_sb[:, c, :], xT[c * 128:(c + 1) * 128, :])
        nc.sync.dma_start(U[:, 0, ds(512 * c, KVD)],
                          Wk[c * 128:(c + 1) * 128, :])
        nc.sync.dma_start(U[:, 0, ds(512 * c + KVD, KVD)],
                          Wv[c * 128:(c + 1) * 128, :])
    for c in range(CD):
        nc.sync.dma_start(Wq_sb[:, c, :], Wq[c * 128:(c + 1) * 128, :])
    for c in range(QD // 128):
        nc.sync.dma_start(Wo_sb[:, c, :], Wo[c * 128:(c + 1) * 128, :])

    nc.vector.memset(ones_b[:, :], 1.0)
    nc.vector.memset(v_sb[:, :, :, DK:DK + 1], 1.0)
    nc.vector.memset(tri[:, :], 1.0)
    nc.gpsimd.affine_select(
        out=tri[:, :], in_=tri[:, :],
        compare_op=mybir.AluOpType.is_ge, fill=0.0,
        base=0, pattern=[[1, 128]], channel_multiplier=-1)

    # ---- K^T and V projections, c-outer in waves ------------------------
    # Matmuls are emitted c-outer so the PE starts as soon as xT chunk 0
    # lands.  Waves 0/1 pair K^T chunk m=p with V token chunks 4p..4p+3
    # (8 PSUM banks); waves 2/3 finish V mt 8..15 densely (xT resident).
    def v_wave(w, with_k):
        vps = ([po.tile([128, KVD], f32, tag="o65", name=f"vp{w}_{i}")
                for i in range(3)]
               + [pp.tile([128, KVD], f32, tag="ps", name=f"vp{w}_3")])
        kps = ([pp.tile([128, TQ], f32, tag="ps", name=f"kp{w}_{n}")
                for n in range(4)] if with_k else None)
        for c in range(CD):
            if with_k:
                for n in range(4):
                    nc.tensor.matmul(kps[n][:, :],
                                     U[:, 0, ds(512 * c + 128 * w, 128)],
                                     xT_sb[:, c, ts(n, TQ)],
                                     start=(c == 0), stop=(c == CD - 1))
            for i in range(4):
                mt = 4 * w + i
                nc.tensor.matmul(vps[i][:, :],
                                 xT_sb[:, c, ts(mt, 128)],
                                 U[:, 0, ds(512 * c + KVD, KVD)],
                                 start=(c == 0), stop=False)
        for i in range(4):
            nc.tensor.matmul(vps[i][:, :], ones_b[:, :],
                             bv_sb[:, :], start=False, stop=True)
        if with_k:
            for n in range(4):
                nc.vector.tensor_scalar_add(kT_sb[:, w, ts(n, TQ)],
                                            kps[n][:, :], bk_sb[:, w:w + 1])
        for i in range(4):
            mt = 4 * w + i
            nc.vector.tensor_copy(
                v_sb[:, mt, :, 0:DK],
                vps[i][:, :].rearrange("p (h d) -> p h d", h=HALF_KV))

    for w in range(4):
        v_wave(w, with_k=(w < 2))

    # buffer 0's zero halves, once K/V-proj has consumed the Wk/Wv bytes
    # hosted there (done once; the zeros persist for the whole kernel)
    for s in range(NQH):
        z0 = (1 - s % 2) * 64
        nc.vector.memset(U[z0:z0 + 64, 0, ds(512 * s, TQ)], 0.0)

    # ---- pipelined per-query-tile main loop -----------------------------
    def qproj_group(jj, m):
        # generator: one PE matmul per next() so it can be paced as filler
        ps = pp.tile([128, TQ], f32, tag="ps", name=f"q{jj}_{m}")
        for c in range(CD):
            nc.tensor.matmul(ps[:, :],
                             Wq_sb[:, c, ts(m, 128)],
                             xT_sb[:, c, ds(jj * TQ, TQ)],
                             start=(c == 0), stop=(c == CD - 1))
            if c < CD - 1:
                yield
        b = jj % 2
        nc.vector.tensor_scalar_add(U[0:64, b, ds(512 * 2 * m, TQ)],
                                    ps[0:64, :], bq_sb[0:64, m:m + 1])
        nc.vector.tensor_scalar_add(U[64:128, b, ds(512 * (2 * m + 1), TQ)],
                                    ps[64:128, :], bq_sb[64:128, m:m + 1])
        yield

    def oproj_group(jj, oT_tile, mt, n):
        ps = pp.tile([128, TQ], f32, tag="ps", name=f"o{jj}_{mt}_{n}")
        for c in range(QD // 128):
            nc.tensor.matmul(ps[:, :],
                             oT_tile[:, c, ts(mt, 128)],
                             Wo_sb[:, c, ts(n, TQ)],
                             start=(c == 0), stop=(c == QD // 128 - 1))
            if c < QD // 128 - 1:
                yield
        os_ = wp.tile([128, TQ], bf16, tag="os", name=f"os{jj}_{mt}_{n}")
        nc.vector.tensor_copy(os_[:, :], ps[:, :])
        nc.sync.dma_start(
            out[ds(jj * TQ + mt * 128, 128), ts(n, TQ)], os_[:, :])
        yield

    def filler_stream(j, oT_tiles):
        # one yield per PE matmul: O-proj of tile j-1, then q-proj of j+1
        if j > 0:
            for mt in range(TQ // 128):
                for n in range(D // TQ):
                    yield from oproj_group(j - 1, oT_tiles[j - 1], mt, n)
        if j < NTQ - 1:
            for m in range(QD // 128):
                yield from qproj_group(j + 1, m)

    oT_tiles = {}
    # prologue: q^T for tile 0 (drain the generators back-to-back)
    for m in range(QD // 128):
        for _ in qproj_group(0, m):
            pass

    for j in range(NTQ):
        oT_sb = op.tile([128, QD // 128, TQ], bf16, tag="oT")
        oT_tiles[j] = oT_sb
        nkeep = 4 * j + 4
        filler = filler_stream(j, oT_tiles)
        n_fill = (128 if j > 0 else 0) + (128 if j < NTQ - 1 else 0) + 24
        n_cks = NQH * nkeep
        fill_acc = 0.0
        fill_rate = n_fill / n_cks

        def fill(k):
            for _ in range(k):
                if next(filler, "done") == "done":
                    break

        for s in range(NQH):
            kv = KV_OF_SLOT[s]
            qsl = U[:, j % 2, ds(512 * s, TQ)]
            o65 = po.tile([65, TQ], f32, tag="o65")
            pTs = {}
            for ck in range(nkeep):
                di = ck - 4 * j
                pT = ptp.tile([128, TQ], bf16, tag="pT")
                if di <= 0:
                    # off-diagonal (or first diagonal) chunk: full width
                    ss = pp.tile([128, TQ], f32, tag="ps")
                    nc.tensor.matmul(ss[:, :],
                                     kT_sb[:, kv // 2, ts(ck, 128)],
                                     qsl[:, :], start=True, stop=True)
                    nc.scalar.activation(pT[:, :], ss[:, :],
                                         mybir.ActivationFunctionType.Exp,
                                         scale=SCALE)
                else:
                    # diagonal chunk di>=1: columns below 128*di are fully
                    # masked -- skip them in the matmul and exp, zero pT
                    w = 128 * di
                    ss = pp.tile([128, TQ], f32, tag="ps")
                    nc.tensor.matmul(ss[:, 0:TQ - w],
                                     kT_sb[:, kv // 2, ts(ck, 128)],
                                     qsl[:, w:TQ], start=True, stop=True)
                    nc.vector.memset(pT[:, 0:w], 0.0)
                    nc.scalar.activation(pT[:, w:TQ], ss[:, 0:TQ - w],
                                         mybir.ActivationFunctionType.Exp,
                                         scale=SCALE)
                if di >= 0:
                    # triangle block: columns [128di, 128di+128)
                    nc.vector.tensor_mul(pT[:, ds(128 * di, 128)],
                                         pT[:, ds(128 * di, 128)],
                                         tri[:, :])
                pTs[ck] = pT
                # skewed PV: consume the previous chunk's probabilities so
                # the PE never waits on this chunk's exp
                if ck > 0:
                    nc.tensor.matmul(o65[:, :],
                                     v_sb[:, ck - 1, kv, :], pTs[ck - 1][:, :],
                                     start=(ck - 1 == 0), stop=False)
                    del pTs[ck - 1]
                fill_acc += fill_rate
                k = int(fill_acc)
                fill_acc -= k
                fill(k)
            nc.tensor.matmul(o65[:, :],
                             v_sb[:, nkeep - 1, kv, :], pTs[nkeep - 1][:, :],
                             start=(nkeep == 1), stop=True)
            del pTs[nkeep - 1]
            # softmax division: 1/sums (row 64) broadcast over the 64
            # o^T rows, fused with the psum->sbuf eviction
            rrow = dvp.tile([1, TQ], f32, tag="rr")
            nc.vector.tensor_copy(rrow[:, :], o65[64:65, :])
            nc.vector.reciprocal_approx_fast(rrow[:, :], rrow[:, :])
            bcs = dvp.tile([64, TQ], f32, tag="bc")
            nc.gpsimd.partition_broadcast(bcs[:, :], rrow[:, :])
            nc.vector.tensor_mul(
                oT_sb[(s % 2) * 64:(s % 2) * 64 + 64, s // 2, :],
                o65[0:64, :], bcs[:, :])
        fill(n_fill)

    # epilogue: O-projection of the last tile
    for mt in range(TQ // 128):
        for n in range(D // TQ):
            for _ in oproj_group(NTQ - 1, oT_tiles[NTQ - 1], mt, n):
                pass


def _build():
    import concourse.mybir as mybir
    import concourse.tile as tile
    from concourse import bacc

    nc = bacc.Bacc("TRN2", target_bir_lowering=False, debug=False,
                   num_devices=NCORES)
    f32, bf16 = mybir.dt.float32, mybir.dt.bfloat16
    aps = {
        "xT": nc.dram_tensor("xT", (D, T), bf16, kind="ExternalInput").ap(),
        "Wq": nc.dram_tensor("Wq", (D, QD), bf16, kind="ExternalInput").ap(),
        "bq": nc.dram_tensor("bq", (QD // 128, 128), f32,
                             kind="ExternalInput").ap(),
        "Wk": nc.dram_tensor("Wk", (D, KVD), bf16, kind="ExternalInput").ap(),
        "bk": nc.dram_tensor("bk", (KVD // 128, 128), f32,
                             kind="ExternalInput").ap(),
        "Wv": nc.dram_tensor("Wv", (D, KVD), bf16, kind="ExternalInput").ap(),
        "bv": nc.dram_tensor("bv", (1, KVD), bf16, kind="ExternalInput").ap(),
        "Wo": nc.dram_tensor("Wo", (QD, D), bf16, kind="ExternalInput").ap(),
        "out": nc.dram_tensor("out", (T, D), bf16, kind="ExternalOutput").ap(),
    }
    with tile.TileContext(nc) as tc:
        with ExitStack() as ctx:
            _body(ctx, tc, aps)
    nc.compile()
    return nc


def _get_nc():
    if "nc" not in _cache:
        _cache["nc"] = _build()
    return _cache["nc"]


def _make_in_maps(x, Wq, bq, Wk, bk, Wv, bv, Wo):
    x = np.asarray(x, np.float32)
    in_maps = []
    for core in range(NCORES):
        b, h = core // 2, core % 2
        Wqh = np.asarray(Wq[:, h * QD:(h + 1) * QD], np.float32)[:, QPERM]
        bqh = np.asarray(bq[h * QD:(h + 1) * QD], np.float32)[QPERM]
        Woh = np.asarray(Wo[h * QD:(h + 1) * QD, :], np.float32)[QPERM, :]
        in_maps.append({
            "xT": np.ascontiguousarray(np.asarray(x[b]).T).astype(BF16),
            "Wq": np.ascontiguousarray(Wqh).astype(BF16),
            "bq": np.ascontiguousarray(bqh.reshape(QD // 128, 128)),
            "Wk": np.asarray(Wk[:, h * KVD:(h + 1) * KVD], np.float32).astype(BF16),
            "bk": np.asarray(bk[h * KVD:(h + 1) * KVD], np.float32).reshape(
                KVD // 128, 128),
            "Wv": np.asarray(Wv[:, h * KVD:(h + 1) * KVD], np.float32).astype(BF16),
            "bv": np.asarray(bv[h * KVD:(h + 1) * KVD], np.float32).reshape(
                1, KVD).astype(BF16),
            "Wo": np.ascontiguousarray(Woh).astype(BF16),
        })
    return in_maps


def kernel(x, Wq, bq, Wk, bk, Wv, bv, Wo, bo, **_):
    from concourse.bass_utils import run_bass_kernel_spmd

    in_maps = _make_in_maps(x, Wq, bq, Wk, bk, Wv, bv, Wo)
    nc = _get_nc()
    res = run_bass_kernel_spmd(nc, in_maps, core_ids=list(range(NCORES)))
    bo = np.asarray(bo, np.float32)
    outs = [np.asarray(res.results[c]["out"], np.float32)
            for c in range(NCORES)]
    return np.stack([outs[2 * b] + outs[2 * b + 1] + bo
                     for b in range(B)], axis=0)

